# revision 1
# baseline (speedup 1.0000x reference)
"""FP8GroupedExperts Trainium2 kernel (expert-parallel over 8 NeuronCores).

Math per expert e (tokens pre-sorted by expert, n_e tokens each):
    h   = silu(x_e @ w1[e].T) * (x_e @ w3[e].T)      # (n_e, HID) SwiGLU
    out = h @ w2[e].T                                 # (n_e, DIM)

Sharding: one expert per core (E == n_cores == 8). Each core's segment is
padded to T tokens; zero rows produce zero outputs (silu(0)*0 == 0) and are
stripped on the host after the gather.

On-chip layout (zero on-chip transposes):
  phase 1 computes h^T tiles (HID on partitions, tokens on free dim):
      y1T = W1 @ x^T  via  matmul(lhsT=W1T chunk, rhs=xT chunk)
      hT  = silu(y1T) * y3T
  phase 2 computes out = h @ W2^T with the HID contraction on partitions:
      out tile = matmul(lhsT=hT chunk, rhs=W2T chunk)
All operands are host-pre-transposed so every DMA is dense.
"""

import sys

for _p in ("/opt/trn_rl_repo",):
    if _p not in sys.path:
        sys.path.append(_p)

import numpy as np
import ml_dtypes

import concourse.bacc as bacc
import concourse.mybir as mybir
import concourse.tile as tile
from concourse.bass import ts
from concourse.bass_utils import run_bass_kernel_spmd

E = 8
DIM = 2048
HID = 5632
T = 2048          # padded tokens per expert (= TOT // E)
P = 128

T_TILE = 512
NT = T // T_TILE            # 4 token tiles
KSUB = DIM // P             # 16 contraction subtiles for phase 1
HSUB = HID // P             # 44 h subtiles (phase-1 outputs / phase-2 contraction)
HG = 4                      # h-subtiles per weight-group DMA
NHG = HSUB // HG            # 11 weight groups
ND = DIM // 512             # 4 output column chunks (phase 2)
NTS = T_TILE // P           # 4 token partition-subtiles (phase 2)

BF16 = mybir.dt.bfloat16
F32 = mybir.dt.float32

_BUILD_CACHE = {}


def _build():
    """Build the per-core Bass module (same NEFF for all 8 cores)."""
    nc = bacc.Bacc(None, target_bir_lowering=False)

    xt = nc.dram_tensor("xt", [DIM, T], BF16, kind="ExternalInput")
    w1t = nc.dram_tensor("w1t", [DIM, HID], BF16, kind="ExternalInput")
    w3t = nc.dram_tensor("w3t", [DIM, HID], BF16, kind="ExternalInput")
    w2t = nc.dram_tensor("w2t", [HID, DIM], BF16, kind="ExternalInput")
    out = nc.dram_tensor("out", [T, DIM], F32, kind="ExternalOutput")

    xt_r = xt.rearrange("(ko p) t -> p ko t", p=P)      # [128, 16, 2048]
    w1_r = w1t.rearrange("(ko p) h -> p ko h", p=P)     # [128, 16, 5632]
    w3_r = w3t.rearrange("(ko p) h -> p ko h", p=P)

    with tile.TileContext(nc) as tc:
        with (
            tc.tile_pool(name="xp", bufs=2) as xp,
            tc.tile_pool(name="wp", bufs=2) as wp,
            tc.tile_pool(name="hp", bufs=1) as hp,
            tc.tile_pool(name="tp", bufs=3) as tp,
            tc.tile_pool(name="w2p", bufs=4) as w2p,
            tc.tile_pool(name="op", bufs=4) as op,
            tc.tile_pool(name="ps1", bufs=2, space="PSUM") as ps1,
            tc.tile_pool(name="ps2", bufs=4, space="PSUM") as ps2,
        ):
            for it in range(NT):
                t0 = it * T_TILE
                xtt = xp.tile([P, KSUB, T_TILE], BF16, tag="xtt")
                nc.sync.dma_start(xtt[:], xt_r[:, :, t0:t0 + T_TILE])
                hsb = hp.tile([P, HSUB, T_TILE], BF16, tag="hsb")

                # ---- phase 1: hT = silu(W1 xT) * (W3 xT), HID on partitions
                for hg in range(NHG):
                    h0 = hg * HG * P
                    w1g = wp.tile([P, KSUB, HG * P], BF16, tag="w1g")
                    nc.sync.dma_start(w1g[:], w1_r[:, :, h0:h0 + HG * P])
                    w3g = wp.tile([P, KSUB, HG * P], BF16, tag="w3g")
                    nc.sync.dma_start(w3g[:], w3_r[:, :, h0:h0 + HG * P])
                    for hh in range(HG):
                        h = hg * HG + hh
                        py1 = ps1.tile([P, T_TILE], F32, tag="py1")
                        for k in range(KSUB):
                            nc.tensor.matmul(
                                py1[:], w1g[:, k, ts(hh, P)], xtt[:, k, :],
                                start=(k == 0), stop=(k == KSUB - 1),
                            )
                        py3 = ps1.tile([P, T_TILE], F32, tag="py3")
                        for k in range(KSUB):
                            nc.tensor.matmul(
                                py3[:], w3g[:, k, ts(hh, P)], xtt[:, k, :],
                                start=(k == 0), stop=(k == KSUB - 1),
                            )
                        smp = tp.tile([P, T_TILE], F32, tag="smp")
                        nc.scalar.activation(
                            smp[:], py1[:], mybir.ActivationFunctionType.Silu
                        )
                        nc.vector.tensor_tensor(
                            hsb[:, h, :], smp[:], py3[:], mybir.AluOpType.mult
                        )

                # ---- phase 2: out tile = hT.T @ W2T, contraction over HID
                for d in range(ND):
                    pos = [
                        ps2.tile([P, 512], F32, tag="po", name=f"po_{i}")
                        for i in range(NTS)
                    ]
                    for h in range(HSUB):
                        w2g = w2p.tile([P, 512], BF16, tag="w2g")
                        nc.sync.dma_start(
                            w2g[:], w2t[h * P:(h + 1) * P, d * 512:(d + 1) * 512]
                        )
                        for i in range(NTS):
                            nc.tensor.matmul(
                                pos[i][:], hsb[:, h, ts(i, P)], w2g[:],
                                start=(h == 0), stop=(h == HSUB - 1),
                            )
                    for i in range(NTS):
                        osb = op.tile([P, 512], F32, tag="osb")
                        nc.vector.tensor_copy(osb[:], pos[i][:])
                        nc.sync.dma_start(
                            out[t0 + i * P:t0 + (i + 1) * P, d * 512:(d + 1) * 512],
                            osb[:],
                        )

    nc.compile()
    return nc


def _get_nc():
    if "nc" not in _BUILD_CACHE:
        _BUILD_CACHE["nc"] = _build()
    return _BUILD_CACHE["nc"]


def _prep_inputs(x, num_tokens_per_expert, w1, w2, w3):
    """Host-side shard + layout prep: per-expert transposed bf16 operands."""
    x = np.asarray(x, dtype=np.float32)
    w1 = np.asarray(w1)
    w2 = np.asarray(w2)
    w3 = np.asarray(w3)
    counts = np.asarray(num_tokens_per_expert).astype(np.int64)
    offs = np.concatenate([[0], np.cumsum(counts)])

    in_maps = []
    for e in range(E):
        n_e = int(counts[e])
        if n_e > T:
            raise ValueError(f"expert {e} has {n_e} tokens > padded capacity {T}")
        xe = x[offs[e]:offs[e] + n_e]
        if n_e < T:
            xe = np.concatenate(
                [xe, np.zeros((T - n_e, DIM), dtype=np.float32)], axis=0
            )
        in_maps.append({
            "xt": np.ascontiguousarray(xe.T).astype(ml_dtypes.bfloat16),
            "w1t": np.ascontiguousarray(np.asarray(w1[e]).T).astype(ml_dtypes.bfloat16),
            "w3t": np.ascontiguousarray(np.asarray(w3[e]).T).astype(ml_dtypes.bfloat16),
            "w2t": np.ascontiguousarray(np.asarray(w2[e]).T).astype(ml_dtypes.bfloat16),
        })
    return in_maps, counts


def _run(inputs, **run_kwargs):
    in_maps, counts = _prep_inputs(
        inputs["x"], inputs["num_tokens_per_expert"],
        inputs["w1"], inputs["w2"], inputs["w3"],
    )
    nc = _get_nc()
    res = run_bass_kernel_spmd(nc, in_maps, core_ids=list(range(E)), **run_kwargs)
    pieces = [res.results[e]["out"][: int(counts[e])] for e in range(E)]
    full = np.concatenate(pieces, axis=0).astype(np.float32)
    return full, res


def kernel(**inputs):
    out, _ = _run(inputs)
    return out


if __name__ == "__main__":
    # Tiny self-check with random data (not the reference inputs).
    rng = np.random.default_rng(0)
    ins = {
        "x": rng.standard_normal((E * T, DIM), dtype=np.float32),
        "num_tokens_per_expert": np.full((E,), T, dtype=np.int64),
        "w1": rng.standard_normal((E, HID, DIM), dtype=np.float32) * 0.02,
        "w2": rng.standard_normal((E, DIM, HID), dtype=np.float32) * 0.02,
        "w3": rng.standard_normal((E, HID, DIM), dtype=np.float32) * 0.02,
    }
    got = kernel(**ins)
    print("out shape:", got.shape, got.dtype)


# revision 3
# speedup vs baseline: 1.0852x; 1.0852x over previous
"""FP8GroupedExperts Trainium2 kernel (expert-parallel over 8 NeuronCores).

Math per expert e (tokens pre-sorted by expert, n_e tokens each):
    h   = silu(x_e @ w1[e].T) * (x_e @ w3[e].T)      # (n_e, HID) SwiGLU
    out = h @ w2[e].T                                 # (n_e, DIM)

Sharding: one expert per core (E == n_cores == 8). Each core's segment is
padded to T tokens; zero rows produce zero outputs (silu(0)*0 == 0) and are
stripped on the host after the gather.

On-chip layout (zero on-chip transposes):
  phase 1 computes h^T tiles (HID on partitions, tokens on free dim):
      y1T = W1 @ x^T  via  matmul(lhsT=W1T chunk, rhs=xT chunk)
      hT  = silu(y1T) * y3T
  phase 2 computes out = h @ W2^T with the HID contraction on partitions:
      out tile = matmul(lhsT=hT chunk, rhs=W2T chunk)
All operands are host-pre-transposed so every DMA is dense.
"""

import sys

for _p in ("/opt/trn_rl_repo",):
    if _p not in sys.path:
        sys.path.append(_p)

import numpy as np
import ml_dtypes

import concourse.bacc as bacc
import concourse.mybir as mybir
import concourse.tile as tile
from concourse.bass import ts
from concourse.bass_utils import run_bass_kernel_spmd

E = 8
DIM = 2048
HID = 5632
T = 2048          # padded tokens per expert (= TOT // E)
P = 128

T_TILE = 512
NT = T // T_TILE            # 4 token tiles
KSUB = DIM // P             # 16 contraction subtiles for phase 1
HSUB = HID // P             # 44 h subtiles (phase-1 outputs / phase-2 contraction)
HG = 4                      # h-subtiles per weight-group DMA
NHG = HSUB // HG            # 11 weight groups
ND = DIM // 512             # 4 output column chunks (phase 2)
NTS = T_TILE // P           # 4 token partition-subtiles (phase 2)

BF16 = mybir.dt.bfloat16
F32 = mybir.dt.float32

_BUILD_CACHE = {}


def _build():
    """Build the per-core Bass module (same NEFF for all 8 cores)."""
    nc = bacc.Bacc(None, target_bir_lowering=False)

    xt = nc.dram_tensor("xt", [DIM, T], BF16, kind="ExternalInput")
    w1t = nc.dram_tensor("w1t", [DIM, HID], BF16, kind="ExternalInput")
    w3t = nc.dram_tensor("w3t", [DIM, HID], BF16, kind="ExternalInput")
    w2t = nc.dram_tensor("w2t", [HID, DIM], BF16, kind="ExternalInput")
    out = nc.dram_tensor("out", [T, DIM], F32, kind="ExternalOutput")

    xt_r = xt.rearrange("(ko p) t -> p ko t", p=P)      # [128, 16, 2048]
    w1_r = w1t.rearrange("(ko p) h -> p ko h", p=P)     # [128, 16, 5632]
    w3_r = w3t.rearrange("(ko p) h -> p ko h", p=P)

    w2_r = w2t.rearrange("(hh p) d -> p hh d", p=P)     # [128, 44, 2048]

    with tile.TileContext(nc) as tc:
        with (
            tc.tile_pool(name="xp", bufs=2) as xp,
            tc.tile_pool(name="wp", bufs=2) as wp,
            tc.tile_pool(name="hp", bufs=1) as hp,
            tc.tile_pool(name="tp", bufs=3) as tp,
            tc.tile_pool(name="w2p", bufs=3) as w2p,
            tc.tile_pool(name="op", bufs=4) as op,
            tc.tile_pool(name="ps1", bufs=1, space="PSUM") as ps1,
            tc.tile_pool(name="ps2", bufs=6, space="PSUM") as ps2,
        ):
            def load_x(it):
                t0 = it * T_TILE
                xtt = xp.tile([P, KSUB, T_TILE], BF16, tag="xtt", name=f"xtt_{it}")
                nc.sync.dma_start(xtt[:], xt_r[:, :, t0:t0 + T_TILE])
                return xtt

            def load_wg(it, hg):
                h0 = hg * HG * P
                w1g = wp.tile(
                    [P, KSUB, HG * P], BF16, tag="w1g", name=f"w1g_{it}_{hg}"
                )
                nc.sync.dma_start(w1g[:], w1_r[:, :, h0:h0 + HG * P])
                w3g = wp.tile(
                    [P, KSUB, HG * P], BF16, tag="w3g", name=f"w3g_{it}_{hg}"
                )
                nc.sync.dma_start(w3g[:], w3_r[:, :, h0:h0 + HG * P])
                return w1g, w3g

            pending = {}
            for it in range(NT):
                t0 = it * T_TILE
                xtt = pending.pop(("x", it), None)
                if xtt is None:
                    xtt = load_x(it)
                hsb = hp.tile([P, HSUB, T_TILE], BF16, tag="hsb")

                # ---- phase 1: hT = silu(W1 xT) * (W3 xT), HID on partitions
                for hg in range(NHG):
                    wg = pending.pop(("w", it, hg), None)
                    if wg is None:
                        wg = load_wg(it, hg)
                    w1g, w3g = wg
                    for hh in range(HG):
                        h = hg * HG + hh
                        py1 = ps1.tile([P, T_TILE], F32, tag="py1")
                        for k in range(KSUB):
                            nc.tensor.matmul(
                                py1[:], w1g[:, k, ts(hh, P)], xtt[:, k, :],
                                start=(k == 0), stop=(k == KSUB - 1),
                            )
                        py3 = ps1.tile([P, T_TILE], F32, tag="py3")
                        for k in range(KSUB):
                            nc.tensor.matmul(
                                py3[:], w3g[:, k, ts(hh, P)], xtt[:, k, :],
                                start=(k == 0), stop=(k == KSUB - 1),
                            )
                        smp = tp.tile([P, T_TILE], F32, tag="smp")
                        nc.scalar.activation(
                            smp[:], py1[:], mybir.ActivationFunctionType.Silu
                        )
                        nc.vector.tensor_tensor(
                            hsb[:, h, :], smp[:], py3[:], mybir.AluOpType.mult
                        )

                # prefetch next iter's activations + first weight group so the
                # sync engine issues them before phase 2's w2 stream
                if it + 1 < NT:
                    pending[("x", it + 1)] = load_x(it + 1)
                    pending[("w", it + 1, 0)] = load_wg(it + 1, 0)

                # ---- phase 2: out tile = hT.T @ W2T, contraction over HID
                W2B = 4  # h-subtiles per w2 DMA
                for d in range(ND):
                    pos = [
                        ps2.tile([P, 512], F32, tag="po", name=f"po_{i}")
                        for i in range(NTS)
                    ]
                    for hb in range(HSUB // W2B):
                        w2g = w2p.tile([P, W2B, 512], BF16, tag="w2g")
                        nc.sync.dma_start(
                            w2g[:],
                            w2_r[:, hb * W2B:(hb + 1) * W2B,
                                 d * 512:(d + 1) * 512],
                        )
                        for hh in range(W2B):
                            h = hb * W2B + hh
                            for i in range(NTS):
                                nc.tensor.matmul(
                                    pos[i][:], hsb[:, h, ts(i, P)], w2g[:, hh, :],
                                    start=(h == 0), stop=(h == HSUB - 1),
                                )
                    for i in range(NTS):
                        osb = op.tile([P, 512], F32, tag="osb")
                        nc.vector.tensor_copy(osb[:], pos[i][:])
                        nc.sync.dma_start(
                            out[t0 + i * P:t0 + (i + 1) * P, d * 512:(d + 1) * 512],
                            osb[:],
                        )

    nc.compile()
    return nc


def _get_nc():
    if "nc" not in _BUILD_CACHE:
        _BUILD_CACHE["nc"] = _build()
    return _BUILD_CACHE["nc"]


def _prep_inputs(x, num_tokens_per_expert, w1, w2, w3):
    """Host-side shard + layout prep: per-expert transposed bf16 operands."""
    x = np.asarray(x, dtype=np.float32)
    w1 = np.asarray(w1)
    w2 = np.asarray(w2)
    w3 = np.asarray(w3)
    counts = np.asarray(num_tokens_per_expert).astype(np.int64)
    offs = np.concatenate([[0], np.cumsum(counts)])

    in_maps = []
    for e in range(E):
        n_e = int(counts[e])
        if n_e > T:
            raise ValueError(f"expert {e} has {n_e} tokens > padded capacity {T}")
        xe = x[offs[e]:offs[e] + n_e]
        if n_e < T:
            xe = np.concatenate(
                [xe, np.zeros((T - n_e, DIM), dtype=np.float32)], axis=0
            )
        in_maps.append({
            "xt": np.ascontiguousarray(xe.T).astype(ml_dtypes.bfloat16),
            "w1t": np.ascontiguousarray(np.asarray(w1[e]).T).astype(ml_dtypes.bfloat16),
            "w3t": np.ascontiguousarray(np.asarray(w3[e]).T).astype(ml_dtypes.bfloat16),
            "w2t": np.ascontiguousarray(np.asarray(w2[e]).T).astype(ml_dtypes.bfloat16),
        })
    return in_maps, counts


def _run(inputs, **run_kwargs):
    in_maps, counts = _prep_inputs(
        inputs["x"], inputs["num_tokens_per_expert"],
        inputs["w1"], inputs["w2"], inputs["w3"],
    )
    nc = _get_nc()
    res = run_bass_kernel_spmd(nc, in_maps, core_ids=list(range(E)), **run_kwargs)
    pieces = [res.results[e]["out"][: int(counts[e])] for e in range(E)]
    full = np.concatenate(pieces, axis=0).astype(np.float32)
    return full, res


def kernel(**inputs):
    out, _ = _run(inputs)
    return out


if __name__ == "__main__":
    # Tiny self-check with random data (not the reference inputs).
    rng = np.random.default_rng(0)
    ins = {
        "x": rng.standard_normal((E * T, DIM), dtype=np.float32),
        "num_tokens_per_expert": np.full((E,), T, dtype=np.int64),
        "w1": rng.standard_normal((E, HID, DIM), dtype=np.float32) * 0.02,
        "w2": rng.standard_normal((E, DIM, HID), dtype=np.float32) * 0.02,
        "w3": rng.standard_normal((E, HID, DIM), dtype=np.float32) * 0.02,
    }
    got = kernel(**ins)
    print("out shape:", got.shape, got.dtype)


# revision 5
# speedup vs baseline: 1.0884x; 1.0029x over previous
"""FP8GroupedExperts Trainium2 kernel (expert-parallel over 8 NeuronCores).

Math per expert e (tokens pre-sorted by expert, n_e tokens each):
    h   = silu(x_e @ w1[e].T) * (x_e @ w3[e].T)      # (n_e, HID) SwiGLU
    out = h @ w2[e].T                                 # (n_e, DIM)

Sharding: one expert per core (E == n_cores == 8). Each core's segment is
padded to T tokens; zero rows produce zero outputs (silu(0)*0 == 0) and are
stripped on the host after the gather.

On-chip layout (zero on-chip transposes):
  phase 1 computes h^T tiles (HID on partitions, tokens on free dim):
      y1T = W1 @ x^T  via  matmul(lhsT=W1T chunk, rhs=xT chunk)
      hT  = silu(y1T) * y3T
  phase 2 computes out = h @ W2^T with the HID contraction on partitions:
      out tile = matmul(lhsT=hT chunk, rhs=W2T chunk)
All operands are host-pre-transposed so every DMA is dense.
"""

import sys

for _p in ("/opt/trn_rl_repo",):
    if _p not in sys.path:
        sys.path.append(_p)

import numpy as np
import ml_dtypes

import concourse.bacc as bacc
import concourse.mybir as mybir
import concourse.tile as tile
from concourse.bass import ts
from concourse.bass_utils import run_bass_kernel_spmd

E = 8
DIM = 2048
HID = 5632
T = 2048          # padded tokens per expert (= TOT // E)
P = 128

T_TILE = 512
NT = T // T_TILE            # 4 token tiles
KSUB = DIM // P             # 16 contraction subtiles for phase 1
HSUB = HID // P             # 44 h subtiles (phase-1 outputs / phase-2 contraction)
HG = 4                      # h-subtiles per weight-group DMA
NHG = HSUB // HG            # 11 weight groups
ND = DIM // 512             # 4 output column chunks (phase 2)
NTS = T_TILE // P           # 4 token partition-subtiles (phase 2)

BF16 = mybir.dt.bfloat16
F32 = mybir.dt.float32

_BUILD_CACHE = {}


def _build():
    """Build the per-core Bass module (same NEFF for all 8 cores)."""
    nc = bacc.Bacc(None, target_bir_lowering=False)

    xt = nc.dram_tensor("xt", [DIM, T], BF16, kind="ExternalInput")
    w1t = nc.dram_tensor("w1t", [DIM, HID], BF16, kind="ExternalInput")
    w3t = nc.dram_tensor("w3t", [DIM, HID], BF16, kind="ExternalInput")
    w2t = nc.dram_tensor("w2t", [HID, DIM], BF16, kind="ExternalInput")
    out = nc.dram_tensor("out", [T, DIM], F32, kind="ExternalOutput")

    xt_r = xt.rearrange("(ko p) t -> p ko t", p=P)      # [128, 16, 2048]
    w1_r = w1t.rearrange("(ko p) h -> p ko h", p=P)     # [128, 16, 5632]
    w3_r = w3t.rearrange("(ko p) h -> p ko h", p=P)

    w2_r = w2t.rearrange("(hh p) d -> p hh d", p=P)     # [128, 44, 2048]

    with tile.TileContext(nc) as tc:
        with (
            tc.tile_pool(name="xp", bufs=2) as xp,
            tc.tile_pool(name="wp", bufs=2) as wp,
            tc.tile_pool(name="hp", bufs=1) as hp,
            tc.tile_pool(name="tp", bufs=3) as tp,
            tc.tile_pool(name="w2p", bufs=3) as w2p,
            tc.tile_pool(name="op", bufs=4) as op,
            tc.tile_pool(name="ps1", bufs=1, space="PSUM") as ps1,
            tc.tile_pool(name="ps2", bufs=6, space="PSUM") as ps2,
        ):
            def load_x(it, split=1):
                # split>1 issues per-k-chunk DMAs so the first matmuls (which
                # only read low k subtiles) can start before the full tile lands
                t0 = it * T_TILE
                xtt = xp.tile([P, KSUB, T_TILE], BF16, tag="xtt", name=f"xtt_{it}")
                kc = KSUB // split
                for s in range(split):
                    nc.sync.dma_start(
                        xtt[:, s * kc:(s + 1) * kc, :],
                        xt_r[:, s * kc:(s + 1) * kc, t0:t0 + T_TILE],
                    )
                return xtt

            def load_wg(it, hg, split=1):
                h0 = hg * HG * P
                w1g = wp.tile(
                    [P, KSUB, HG * P], BF16, tag="w1g", name=f"w1g_{it}_{hg}"
                )
                w3g = wp.tile(
                    [P, KSUB, HG * P], BF16, tag="w3g", name=f"w3g_{it}_{hg}"
                )
                kc = KSUB // split
                for s in range(split):
                    ks = slice(s * kc, (s + 1) * kc)
                    nc.sync.dma_start(w1g[:, ks, :], w1_r[:, ks, h0:h0 + HG * P])
                    nc.sync.dma_start(w3g[:, ks, :], w3_r[:, ks, h0:h0 + HG * P])
                return w1g, w3g

            pending = {}
            for it in range(NT):
                t0 = it * T_TILE
                xtt = pending.pop(("x", it), None)
                if xtt is None:
                    xtt = load_x(it, split=4 if it == 0 else 1)
                hsb = hp.tile([P, HSUB, T_TILE], BF16, tag="hsb")

                # ---- phase 1: hT = silu(W1 xT) * (W3 xT), HID on partitions
                for hg in range(NHG):
                    wg = pending.pop(("w", it, hg), None)
                    if wg is None:
                        wg = load_wg(it, hg, split=4 if (it == 0 and hg == 0) else 1)
                    w1g, w3g = wg
                    for hh in range(HG):
                        h = hg * HG + hh
                        py1 = ps1.tile([P, T_TILE], F32, tag="py1")
                        for k in range(KSUB):
                            nc.tensor.matmul(
                                py1[:], w1g[:, k, ts(hh, P)], xtt[:, k, :],
                                start=(k == 0), stop=(k == KSUB - 1),
                            )
                        py3 = ps1.tile([P, T_TILE], F32, tag="py3")
                        for k in range(KSUB):
                            nc.tensor.matmul(
                                py3[:], w3g[:, k, ts(hh, P)], xtt[:, k, :],
                                start=(k == 0), stop=(k == KSUB - 1),
                            )
                        smp = tp.tile([P, T_TILE], F32, tag="smp")
                        nc.scalar.activation(
                            smp[:], py1[:], mybir.ActivationFunctionType.Silu
                        )
                        nc.vector.tensor_tensor(
                            hsb[:, h, :], smp[:], py3[:], mybir.AluOpType.mult
                        )

                # prefetch next iter's activations + first weight group so the
                # sync engine issues them before phase 2's w2 stream
                if it + 1 < NT:
                    pending[("x", it + 1)] = load_x(it + 1)
                    pending[("w", it + 1, 0)] = load_wg(it + 1, 0)

                # ---- phase 2: out tile = hT.T @ W2T, contraction over HID
                W2B = 4  # h-subtiles per w2 DMA
                for d in range(ND):
                    pos = [
                        ps2.tile([P, 512], F32, tag="po", name=f"po_{i}")
                        for i in range(NTS)
                    ]
                    for hb in range(HSUB // W2B):
                        w2g = w2p.tile([P, W2B, 512], BF16, tag="w2g")
                        nc.sync.dma_start(
                            w2g[:],
                            w2_r[:, hb * W2B:(hb + 1) * W2B,
                                 d * 512:(d + 1) * 512],
                        )
                        for hh in range(W2B):
                            h = hb * W2B + hh
                            for i in range(NTS):
                                nc.tensor.matmul(
                                    pos[i][:], hsb[:, h, ts(i, P)], w2g[:, hh, :],
                                    start=(h == 0), stop=(h == HSUB - 1),
                                )
                    for i in range(NTS):
                        osb = op.tile([P, 512], F32, tag="osb")
                        nc.vector.tensor_copy(osb[:], pos[i][:])
                        nc.sync.dma_start(
                            out[t0 + i * P:t0 + (i + 1) * P, d * 512:(d + 1) * 512],
                            osb[:],
                        )

    nc.compile()
    return nc


def _get_nc():
    if "nc" not in _BUILD_CACHE:
        _BUILD_CACHE["nc"] = _build()
    return _BUILD_CACHE["nc"]


def _prep_inputs(x, num_tokens_per_expert, w1, w2, w3):
    """Host-side shard + layout prep: per-expert transposed bf16 operands."""
    x = np.asarray(x, dtype=np.float32)
    w1 = np.asarray(w1)
    w2 = np.asarray(w2)
    w3 = np.asarray(w3)
    counts = np.asarray(num_tokens_per_expert).astype(np.int64)
    offs = np.concatenate([[0], np.cumsum(counts)])

    in_maps = []
    for e in range(E):
        n_e = int(counts[e])
        if n_e > T:
            raise ValueError(f"expert {e} has {n_e} tokens > padded capacity {T}")
        xe = x[offs[e]:offs[e] + n_e]
        if n_e < T:
            xe = np.concatenate(
                [xe, np.zeros((T - n_e, DIM), dtype=np.float32)], axis=0
            )
        in_maps.append({
            "xt": np.ascontiguousarray(xe.T).astype(ml_dtypes.bfloat16),
            "w1t": np.ascontiguousarray(np.asarray(w1[e]).T).astype(ml_dtypes.bfloat16),
            "w3t": np.ascontiguousarray(np.asarray(w3[e]).T).astype(ml_dtypes.bfloat16),
            "w2t": np.ascontiguousarray(np.asarray(w2[e]).T).astype(ml_dtypes.bfloat16),
        })
    return in_maps, counts


def _run(inputs, **run_kwargs):
    in_maps, counts = _prep_inputs(
        inputs["x"], inputs["num_tokens_per_expert"],
        inputs["w1"], inputs["w2"], inputs["w3"],
    )
    nc = _get_nc()
    res = run_bass_kernel_spmd(nc, in_maps, core_ids=list(range(E)), **run_kwargs)
    pieces = [res.results[e]["out"][: int(counts[e])] for e in range(E)]
    full = np.concatenate(pieces, axis=0).astype(np.float32)
    return full, res


def kernel(**inputs):
    out, _ = _run(inputs)
    return out


if __name__ == "__main__":
    # Tiny self-check with random data (not the reference inputs).
    rng = np.random.default_rng(0)
    ins = {
        "x": rng.standard_normal((E * T, DIM), dtype=np.float32),
        "num_tokens_per_expert": np.full((E,), T, dtype=np.int64),
        "w1": rng.standard_normal((E, HID, DIM), dtype=np.float32) * 0.02,
        "w2": rng.standard_normal((E, DIM, HID), dtype=np.float32) * 0.02,
        "w3": rng.standard_normal((E, HID, DIM), dtype=np.float32) * 0.02,
    }
    got = kernel(**ins)
    print("out shape:", got.shape, got.dtype)
